# revision 28
# baseline (speedup 1.0000x reference)
"""Trainium2 Bass kernel: 8-expert top-2 MoE MLP (SwiGLU), expert-parallel on 8 cores.

Strategy (per sharding hint, expert-parallel):
  - Host: router matmul + top-2 + softmax weights (67 MFLOP — negligible),
    gather each expert's tokens into a zero-padded capacity-C buffer, staged
    TRANSPOSED and chunk-major so every device DMA is one contiguous 2D copy.
  - Device (per core = one expert): fused SwiGLU FFN as two chained GEMMs with
    features on partitions and tokens on the moving free dim:
      H'^T[2M, C] = (W13 stationary).T-free x X^T moving  (contract D)
      H^T = silu(gate) * up                               (ACT + DVE)
      O^T[D, C]  = (W2 stationary) x H^T moving           (contract M)
  - Host: weighted scatter-add of the 8 per-expert outputs back to token order.

Weights live in SBUF for the whole kernel (12 MB bf16/core). All matmuls are
bf16 with fp32 PSUM accumulation (rel err ~5e-3 vs fp32 reference).

DMA issue order/count is the critical path (~0.6us per dma_start on the issuing
engine, transfers drain in issue order at ~300 GB/s): x block 0 (1 DMA), then
W13 in 8 chunk-major DMAs (the first 1 MB chunk unblocks mm1's j-loop), then
the remaining x blocks and W2 hidden behind block-0 compute.
"""

from contextlib import ExitStack

import ml_dtypes
import numpy as np

import concourse.bass as bass  # noqa: F401  (AP helpers)
import concourse.tile as tile
from concourse import bacc, mybir
from concourse.bass_utils import run_bass_kernel_spmd

# nn_MoEMLP_82617990905863 (hardcoded per contract)
B, S, D = 4, 2048, 1024
T = B * S               # 8192 tokens
E = 8                   # experts == cores
TOPK = 2
M = 2048                # MOE_DIM (w13 = [D, 2M], w2 = [M, D])
TB = 512                # token block = max moving free dim
KD = D // 128           # 8 contraction tiles for X @ W13
KH = M // 128           # 16 contraction tiles for H @ W2
# w13 chunk schedule: chunk i covers hidden-col tiles W13_CHUNKS[i] (j indices).
# The first two chunks are single-j (0.5 MB) so the first matmul group is
# unblocked after ~1 MB of transfer; the rest are 2-j (1 MB).
W13_CHUNKS = [[0], [1]] + [[j, j + 1] for j in range(2, 16, 2)]

_NC_CACHE: dict[int, object] = {}
last_results = None     # BassKernelResults of the most recent run (for test.py)


def _blocks(C: int) -> list[int]:
    """Token-block plan: 512s then remainder. 512 is also the MINIMUM viable
    block width for mm1: the j-loop consumes w13 at 1 MB per 16*(N/2.4+2.5) ns
    and the HBM stream delivers ~0.33 MB/us — any N < ~480 starves block 0."""
    sizes = []
    left = C
    while left > TB:
        sizes.append(TB)
        left -= TB
    if left:
        sizes.append(left)
    return sizes


def _build(C: int, use_silu: bool = True, out_bf16: bool = True):
    """Build + compile the SPMD per-core graph for capacity C (even).

    use_silu=False decomposes silu as g*sigmoid(g) (CoreSim lacks the Silu LUT).
    """
    dt = mybir.dt
    odt = dt.bfloat16 if out_bf16 else dt.float32
    nc = bacc.Bacc(
        "TRN2", target_bir_lowering=False, debug=False, enable_asserts=False
    )
    sizes = _blocks(C)
    offs = [0]
    for s in sizes:
        offs.append(offs[-1] + s)
    nblk = len(sizes)
    # chunk-major host layouts — each DMA below is a contiguous [128, W] copy:
    #   xt : [p, block, k, tok]     w13: [p, chunk, k, g|u]     w2: [p, k, d]
    xt = nc.dram_tensor("xt", [128, KD * C], dt.bfloat16, kind="ExternalInput").ap()
    # x block 0 + w13 chunk 0 interleaved by k-pair: one DMA per pair covers
    # both operands of matmuls k=2q,2q+1 of the first group (fewer DMAs on
    # the critical ramp; each carries its own gating semaphore)
    xw0 = nc.dram_tensor(
        "xw0", [128, 4 * (2 * 512 + 512)], dt.bfloat16, kind="ExternalInput"
    ).ap()
    w13 = nc.dram_tensor(
        "w13", [128, KD * 2 * M], dt.bfloat16, kind="ExternalInput"
    ).ap()
    w2 = nc.dram_tensor("w2", [128, KH * D], dt.bfloat16, kind="ExternalInput").ap()
    # output staged like xt: block-major, [d][tok] within a block (host unstages)
    ot = nc.dram_tensor("ot", [128, KD * C], odt, kind="ExternalOutput").ap()

    with tile.TileContext(nc) as tc, ExitStack() as ctx:
        wpool = ctx.enter_context(tc.tile_pool(name="w", bufs=1))
        xpool = ctx.enter_context(tc.tile_pool(name="x", bufs=1))
        spool = ctx.enter_context(tc.tile_pool(name="s", bufs=6))
        hpool = ctx.enter_context(tc.tile_pool(name="h", bufs=2))
        opool = ctx.enter_context(tc.tile_pool(name="o", bufs=2))
        pg = ctx.enter_context(tc.tile_pool(name="pg", bufs=3, space="PSUM"))
        pu = ctx.enter_context(tc.tile_pool(name="pu", bufs=3, space="PSUM"))
        po = ctx.enter_context(tc.tile_pool(name="po", bufs=2, space="PSUM"))

        def x_off(b):
            return KD * offs[b]

        # 1) x block 0 and w13 chunk 0, both in per-k slices interleaved, so
        #    the k-th matmul of the first group is gated on only ~0.2 MB more
        #    of transfer than the (k-1)-th
        xb = [None] * nblk
        n0 = sizes[0]
        x0k = []

        wt = []
        j_chunk = {}            # j -> (chunk idx, local jj, cgw)
        w13_offs = []
        off = 0
        for ci, js in enumerate(W13_CHUNKS):
            w13_offs.append(off)
            for jj, j in enumerate(js):
                j_chunk[j] = (ci, jj, 128 * len(js))
            off += KD * 2 * 128 * len(js)

        def load_w13_chunk(ci):
            js = W13_CHUNKS[ci]
            cgw = 128 * len(js)
            t = wpool.tile([128, KD * 2 * cgw], dt.bfloat16, tag=f"wc{ci}")
            nc.sync.dma_start(
                t[:], w13[:, w13_offs[ci] : w13_offs[ci] + KD * 2 * cgw]
            )
            return t

        # Warm the PE clock (HAM un-throttle needs ~3.4us of sustained PE
        # activity) with throwaway matmuls on a zeroed tile while the first
        # real DMAs are still in flight — the real matmuls then start at
        # 2.4 GHz instead of 1.2.
        warm = xpool.tile([128, 512], dt.bfloat16, tag="warm")
        nc.vector.memset(warm[:], 0)
        wpsum = po.tile([128, 512], dt.float32, tag="po")
        for _ in range(8):
            nc.tensor.matmul(
                wpsum[:], warm[:, 0:128], warm[:, 0:512], start=True, stop=True
            )

        # x block 0 + w13 chunk 0 as 4 combined k-pair tiles: matmul k of the
        # first group is gated on ~0.4 MB (pair 0) .. 1.6 MB (pair 3) of
        # transfer. w13 chunk 1 (j=1) is issued mid-way so the DMA queue
        # delivers it before the j=1 group starts.
        for q in range(4):
            t = xpool.tile([128, 1536], dt.bfloat16, tag=f"xw0q{q}")
            nc.sync.dma_start(t[:], xw0[:, q * 1536 : (q + 1) * 1536])
            x0k.append(t)
            if q == 1:
                wc1 = load_w13_chunk(1)
        wt.append(None)         # chunk 0 handled via the xw0 pair tiles
        wt.append(wc1)
        for ci in range(2, len(W13_CHUNKS)):
            wt.append(load_w13_chunk(ci))

        # 3) rest of x (needed from ~50us) then W2 (needed ~45us)
        for b in range(1, nblk):
            n = sizes[b]
            xb_tile = xpool.tile([128, KD * n], dt.bfloat16, tag=f"xb{b}")
            xb[b] = xb_tile
            nc.sync.dma_start(xb_tile[:], xt[:, x_off(b) : x_off(b) + KD * n])
        w2t = wpool.tile([128, KH * D], dt.bfloat16, tag="w2")
        nc.sync.dma_start(w2t[:], w2[:, :])

        def x_slice(b, k, n):
            if b == 0:
                q, kk = divmod(k, 2)
                return x0k[q][:, kk * 512 : kk * 512 + n]
            return xb[b][:, k * n : (k + 1) * n]

        def w13_slice(ci, jj, cgw, k, gate):
            if ci == 0:
                q, kk = divmod(k, 2)
                base = 1024 + kk * 256 + (0 if gate else 128)
                return x0k[q][:, base : base + 128]
            base = k * 2 * cgw + (0 if gate else cgw) + jj * 128
            return wt[ci][:, base : base + 128]

        for b in range(nblk):
            n = sizes[b]
            h_t = []
            for j in range(KH):
                ci, jj, cgw = j_chunk[j]
                g = pg.tile([128, n], dt.float32, tag="pg")
                u = pu.tile([128, n], dt.float32, tag="pu")

                def mm1(ks, gate, first, last):
                    p = g if gate else u
                    for k in ks:
                        nc.tensor.matmul(
                            p[:],
                            w13_slice(ci, jj, cgw, k, gate),
                            x_slice(b, k, n),
                            start=(k == first),
                            stop=(k == last),
                        )

                if b == 0 and j == 0:
                    # k6/k7 data (xw0 pair 3) lands last on the DMA-bound
                    # ramp — run both groups' k0-5 while waiting for it
                    mm1(range(6), True, 0, KD - 1)
                    mm1(range(6), False, 0, KD - 1)
                    mm1((6, 7), True, 0, KD - 1)
                    mm1((6, 7), False, 0, KD - 1)
                else:
                    mm1(range(KD), True, 0, KD - 1)
                    mm1(range(KD), False, 0, KD - 1)
                gs = spool.tile([128, n], dt.float32, tag="gs")
                if use_silu:
                    nc.scalar.activation(
                        gs[:], g[:], mybir.ActivationFunctionType.Silu
                    )
                else:
                    sg = spool.tile([128, n], dt.float32, tag="sg")
                    nc.scalar.activation(
                        sg[:], g[:], mybir.ActivationFunctionType.Sigmoid
                    )
                    nc.vector.tensor_mul(gs[:], g[:], sg[:])
                h = hpool.tile([128, n], dt.bfloat16, tag=f"h{j}")
                nc.vector.tensor_mul(h[:], gs[:], u[:])
                h_t.append(h)
            ob = opool.tile([128, (KD - 1) * n], odt, tag="o")
            o7 = opool.tile([128, n], odt, tag="o7")
            for d in range(KD):
                p = po.tile([128, n], dt.float32, tag="po")
                for j in range(KH):
                    nc.tensor.matmul(
                        p[:],
                        w2t[:, j * D + d * 128 : j * D + (d + 1) * 128],
                        h_t[j][:],
                        start=(j == 0),
                        stop=(j == KH - 1),
                    )
                if d < KD - 1:
                    nc.vector.tensor_copy(ob[:, d * n : (d + 1) * n], p[:])
                else:
                    nc.vector.tensor_copy(o7[:], p[:])
                if d == KD - 2:
                    # d0-d6 in one DMA while d7 computes; d7 alone after its
                    # cast so the final transfer (and teardown) starts ASAP
                    nc.sync.dma_start(
                        ot[:, x_off(b) : x_off(b) + (KD - 1) * n], ob[:]
                    )
            # issue from the (idle) scalar queue so it doesn't serialize
            # behind the d0-6 issue on sync — this transfer gates teardown
            nc.scalar.dma_start(
                ot[:, x_off(b) + (KD - 1) * n : x_off(b) + KD * n], o7[:]
            )

    nc.compile()
    return nc


def _stage_x(xg: np.ndarray) -> np.ndarray:
    """[C, D] gathered tokens -> [128, block-major (b, k, tok)] bf16."""
    C = xg.shape[0]
    a = np.ascontiguousarray(xg.T).reshape(KD, 128, C)       # [k, p, tok]
    blocks = []
    c0 = 0
    for n in _blocks(C):
        blocks.append(a[:, :, c0 : c0 + n].transpose(1, 0, 2).reshape(128, KD * n))
        c0 += n
    return np.ascontiguousarray(np.concatenate(blocks, axis=1))


def _stage_w13(w: np.ndarray) -> np.ndarray:
    """[D, 2M] gate|up -> [128, chunk-major (chunk, k, g|u)] bf16."""
    parts = []
    for js in W13_CHUNKS:
        cgw = 128 * len(js)
        cols_g = np.concatenate([w[:, j * 128 : (j + 1) * 128] for j in js], axis=1)
        cols_u = np.concatenate(
            [w[:, M + j * 128 : M + (j + 1) * 128] for j in js], axis=1
        )
        a = np.concatenate([cols_g, cols_u], axis=1)         # [D, 2cgw]
        parts.append(
            a.reshape(KD, 128, 2 * cgw).transpose(1, 0, 2).reshape(128, KD * 2 * cgw)
        )
    return np.ascontiguousarray(np.concatenate(parts, axis=1))


def _stage_w2(w: np.ndarray) -> np.ndarray:
    """[M, D] -> [128, (k, d)] bf16."""
    return np.ascontiguousarray(
        w.reshape(KH, 128, D).transpose(1, 0, 2).reshape(128, KH * D)
    )


def _unstage_o(ote: np.ndarray, C: int) -> np.ndarray:
    """[128, block-major (b, d, tok)] -> [D, C] (inverse of the x staging)."""
    blocks = []
    c0 = 0
    for n in _blocks(C):
        blk = ote[:, KD * c0 : KD * c0 + KD * n].reshape(128, KD, n)
        blocks.append(blk.transpose(1, 0, 2).reshape(D, n))
        c0 += n
    return np.concatenate(blocks, axis=1)


def _route(xf: np.ndarray, moe_router: np.ndarray):
    """Top-2 routing on host. Returns per-expert (rows, weights)."""
    logits = xf @ moe_router                      # [T, E] f32
    top1 = np.argmax(logits, axis=1)
    tmp = logits.copy()
    tmp[np.arange(T), top1] = -np.inf
    top2 = np.argmax(tmp, axis=1)
    l1 = logits[np.arange(T), top1]
    l2 = logits[np.arange(T), top2]
    mx = np.maximum(l1, l2)
    e1 = np.exp(l1 - mx)
    e2 = np.exp(l2 - mx)
    s = e1 + e2
    w1 = (e1 / s).astype(np.float32)
    w2 = (e2 / s).astype(np.float32)
    per_expert = []
    for e in range(E):
        r1 = np.where(top1 == e)[0]
        r2 = np.where(top2 == e)[0]
        rows = np.concatenate([r1, r2])
        wts = np.concatenate([w1[r1], w2[r2]]).astype(np.float32)
        per_expert.append((rows, wts))
    return per_expert


def kernel(x, moe_router, moe_w13, moe_w2, _trace=False, _trace_kwargs=None):
    global last_results
    x = np.asarray(x)
    moe_router = np.asarray(moe_router)
    moe_w13 = np.asarray(moe_w13)
    moe_w2 = np.asarray(moe_w2)
    xf = np.ascontiguousarray(x.reshape(T, D).astype(np.float32))
    per_expert = _route(xf, np.asarray(moe_router, dtype=np.float32))

    cmax = max(len(rows) for rows, _ in per_expert)
    C = cmax + (cmax & 1)       # even, else exact (padding is pure overhead)
    C = max(C, 2 * TB)          # keep the block-0 / rest split well-formed

    nc = _NC_CACHE.get(C)
    if nc is None:
        nc = _build(C)
        _NC_CACHE[C] = nc

    xf_bf = xf.astype(ml_dtypes.bfloat16)
    in_maps = []
    for e in range(E):
        rows, _ = per_expert[e]
        xg = np.zeros((C, D), dtype=ml_dtypes.bfloat16)
        xg[: len(rows)] = xf_bf[rows]
        xt_s = _stage_x(xg)
        w13_s = _stage_w13(np.asarray(moe_w13[e]).astype(ml_dtypes.bfloat16))
        xw0 = np.concatenate(
            [
                np.concatenate(
                    [
                        xt_s[:, q * 1024 : (q + 1) * 1024],
                        w13_s[:, q * 512 : (q + 1) * 512],
                    ],
                    axis=1,
                )
                for q in range(4)
            ],
            axis=1,
        )
        in_maps.append(
            {
                "xt": xt_s,
                "xw0": np.ascontiguousarray(xw0),
                "w13": w13_s,
                "w2": _stage_w2(np.asarray(moe_w2[e]).astype(ml_dtypes.bfloat16)),
            }
        )

    res = run_bass_kernel_spmd(
        nc,
        in_maps,
        core_ids=list(range(E)),
        trace=_trace,
        **(_trace_kwargs or {}),
    )
    last_results = res

    out = np.zeros((T, D), dtype=np.float32)
    for e in range(E):
        rows, wts = per_expert[e]
        ote = _unstage_o(np.asarray(res.results[e]["ot"]), C)   # [D, C]
        out[rows] += ote[:, : len(rows)].T.astype(np.float32) * wts[:, None]
    return out.reshape(B, S, D)

